# revision 3
# baseline (speedup 1.0000x reference)
"""Multi-head attention (B=1, L=4096, D=1024, H=16, d_k=64) on 8 TRN2 NeuronCores.

Sharding: head/tensor parallel. Core c owns heads 2c, 2c+1 (128 dmodel dims):
its slices of the QKV projection weights, full-L attention for its 2 heads,
and the partial O-projection for its 128-dim slice. Host sums the 8 partial
outputs (the allreduce of row-sharded tensor parallelism, done at gather).

v2 vs v1:
- ALL matmuls fp16 (v1 ran attention in f32r -> fp32_mode=HIGH on the PE,
  which power-throttled HAM to 1.2 GHz for ~440us of the run, and disabled
  FWL so every LDWEIGHTS cost ~210ns serialized). fp16 keeps K=8/8, enables
  FWL (hidden weight loads), and halves SBUF/stream energy.
- Score matmuls for the two heads are emitted adjacently as 64x128 row-tiles
  (T0/T8) so they co-issue and stream concurrently: 2x on scores.
- AV stationary is [V_h | ones64] ([ones64 | V_h] for head 1), so the softmax
  denominator comes out of the AV matmul already broadcast across 64
  partitions (the array columns were idle anyway). Normalization is then
  full-width DVE ops; no [1,512] reciprocal, no gpsimd broadcast.
- The two denominator blocks land in opposite partition halves from their AV
  blocks; two tiny 64x64 identity matmuls (T8/T2) swap the halves on the PE.
- O-projection contracts the full 128 local dims in one matmul per
  (dmodel-tile, q-chunk) instead of two 64-contractions.
- exp tiles are [128, 3*512] (GK=3) to amortize the ACT engine's ~352-cycle
  per-instruction overhead; scalar-engine exp is the critical path (~250us).
- fp16 output (halves the output DMA); host gathers/sums in fp32.
"""
import sys
if '/opt/trn_rl_repo' not in sys.path:
    sys.path.insert(0, '/opt/trn_rl_repo')

import numpy as np
from contextlib import ExitStack

import concourse.bass as bass
import concourse.tile as tile
from concourse import bacc, mybir
from concourse._compat import with_exitstack
from concourse.bass_utils import run_bass_kernel_spmd
from concourse.masks import make_identity

F32 = mybir.dt.float32
F16 = mybir.dt.float16
AF = mybir.ActivationFunctionType

N_CORES = 8
L = 4096
D = 1024
QC = 512                 # q-chunk (free dim of score/AV/O matmuls)
NQC = L // QC            # 8
NKT = L // 128           # 32 k-tiles
NDT = D // 128           # 8 dmodel tiles
GK = 3                   # k-tiles per exp group ([128, GK*QC] ACT tiles)
SCALE = 0.125            # 1/sqrt(d_k)


def _groups():
    """Split NKT k-tiles into groups of GK (last group ragged)."""
    gs, k0 = [], 0
    while k0 < NKT:
        nk = min(GK, NKT - k0)
        gs.append((k0, nk))
        k0 += nk
    return gs


@with_exitstack
def _mha_core_kernel(ctx, tc, outs, ins, reps=1):
    sb = ctx.enter_context(tc.tile_pool(name="sb", bufs=1))
    for _rep in range(reps):
        _mha_body(tc, sb, outs, ins)


def _mha_body(tc, sb, outs, ins):
    nc = tc.nc
    (outT,) = outs           # [NDT, NQC, 128, QC] fp16
    qT, kT, vT, wq, wk, wv, wo = ins
    # qT/kT/vT: [D, L] fp16 transposed activations
    # wq/wk/wv: [128, NDT, 128] fp16 (lhsT per ktile)
    # wo:       [128, NDT, 128] fp16 (lhsT per mtile; rows = local dims)

    # ---- weights (single contiguous DMAs) ----
    wq_s = sb.tile([128, NDT, 128], F16, tag="wq")
    wk_s = sb.tile([128, NDT, 128], F16, tag="wk")
    wv_s = sb.tile([128, NDT, 128], F16, tag="wv")
    wo_s = sb.tile([128, NDT, 128], F16, tag="wo")
    for w_s, w_d in ((wq_s, wq), (wk_s, wk), (wv_s, wv), (wo_s, wo)):
        nc.sync.dma_start(w_s[:], w_d[:])

    ident = sb.tile([128, 128], F16, tag="ident")
    make_identity(nc, ident[:])

    # ---- persistent activations (all fp16) ----
    QT_s = sb.tile([128, L], F16, tag="QT")
    KT_s = sb.tile([128, L], F16, tag="KT")
    VT_s = sb.tile([128, L], F16, tag="VT")
    # Vaug per ktile: cols [0:64]=V_h0, [64:128]=1.0  (head0 aug = [V0|1])
    #                 cols [128:192]=1.0, [192:256]=V_h1 (head1 aug = [1|V1])
    # => AV_h0 lands in psum rows 0:64 with d0 broadcast in rows 64:128;
    #    AV_h1 lands in rows 64:128 with d1 broadcast in rows 0:64.
    Vaug = sb.tile([128, NKT, 256], F16, tag="Vaug")
    nc.vector.memset(Vaug[:, :, 64:192], 1.0)

    # ========== phase 1: K, Q, V projections (t-major 1 MiB streams) ==========
    with (
        tc.tile_pool(name="xblk", bufs=3) as xblk,
        tc.tile_pool(name="pacc", bufs=1, space="PSUM") as pacc,
    ):
        def proj(dst, w_s, src_d):
            accs = [pacc.tile([128, QC], F32, tag=f"acc{qc}", name=f"acc{qc}")
                    for qc in range(NQC)]
            for t in range(NDT):
                blk = xblk.tile([128, L], F16, tag="blk")
                nc.sync.dma_start(blk[:], src_d[t * 128:(t + 1) * 128, :])
                for qc in range(NQC):
                    nc.tensor.matmul(accs[qc][:], w_s[:, t, :],
                                     blk[:, qc * QC:(qc + 1) * QC],
                                     start=(t == 0), stop=(t == NDT - 1))
            for qc in range(NQC):
                nc.vector.tensor_copy(dst[:, qc * QC:(qc + 1) * QC], accs[qc][:])

        proj(KT_s, wk_s, kT)
        proj(QT_s, wq_s, qT)
        proj(VT_s, wv_s, vT)

    # transpose VT -> Vaug rows (L-major), heads split around the ones blocks
    with tc.tile_pool(name="tps", bufs=2, space="PSUM") as tps:
        for rt in range(NKT):
            tp = tps.tile([128, 128], F16, tag="tp")
            nc.tensor.transpose(tp[:], VT_s[:, rt * 128:(rt + 1) * 128], ident[:])
            nc.vector.tensor_copy(Vaug[:, rt, 0:64], tp[:, 0:64])
            nc.vector.tensor_copy(Vaug[:, rt, 192:256], tp[:, 64:128])

    # ====== phase 2: per q-chunk: attention + O-proj ======
    groups = _groups()
    with (
        tc.tile_pool(name="pst", bufs=2, space="PSUM") as pst,
        tc.tile_pool(name="pot0", bufs=1, space="PSUM") as pot0,
        tc.tile_pool(name="pot1", bufs=1, space="PSUM") as pot1,
        tc.tile_pool(name="pat", bufs=2) as pat,
        tc.tile_pool(name="psm", bufs=2) as psm,
        tc.tile_pool(name="poc", bufs=2) as poc,
    ):
        for qc in range(NQC):
            q0, q1 = qc * QC, (qc + 1) * QC
            ot0 = pot0.tile([128, QC], F32, tag="ot0")
            ot1 = pot1.tile([128, QC], F32, tag="ot1")
            for (k0, nk) in groups:
                st0 = pst.tile([128, GK * QC], F32, tag="st", name="st0")
                st1 = pst.tile([128, GK * QC], F32, tag="st", name="st1")
                # scores: the two heads' matmuls are 64x128 row-tiles
                # (T0/T8, auto tile_position from base partitions) emitted
                # adjacently so they stream concurrently.
                for j in range(nk):
                    kt = k0 + j
                    ks = slice(kt * 128, (kt + 1) * 128)
                    js = slice(j * QC, (j + 1) * QC)
                    nc.tensor.matmul(st0[:, js], KT_s[0:64, ks],
                                     QT_s[0:64, q0:q1], start=True, stop=True)
                    nc.tensor.matmul(st1[:, js], KT_s[64:128, ks],
                                     QT_s[64:128, q0:q1], start=True, stop=True)
                gs = slice(0, nk * QC)
                at0 = pat.tile([128, GK * QC], F16, tag="at0")
                nc.scalar.activation(at0[:, gs], st0[:, gs], AF.Exp, scale=SCALE)
                at1 = pat.tile([128, GK * QC], F16, tag="at1")
                nc.scalar.activation(at1[:, gs], st1[:, gs], AF.Exp, scale=SCALE)
                for j in range(nk):
                    kt = k0 + j
                    js = slice(j * QC, (j + 1) * QC)
                    nc.tensor.matmul(ot0[:], Vaug[:, kt, 0:128], at0[:, js],
                                     start=(kt == 0), stop=(kt == NKT - 1))
                    nc.tensor.matmul(ot1[:], Vaug[:, kt, 128:256], at1[:, js],
                                     start=(kt == 0), stop=(kt == NKT - 1))

            # -- normalize: swap the denominator halves via two 64x64
            #    identity matmuls (T8/T2), then one reciprocal + two muls.
            dstack = psm.tile([128, QC], F16, tag="dstack")
            nc.vector.tensor_copy(dstack[64:128, :], ot0[64:128, :])   # d0
            nc.vector.tensor_copy(dstack[0:64, :], ot1[0:64, :])       # d1
            mv = pst.tile([128, QC], F32, tag="st", name="mv")
            nc.tensor.matmul(mv[0:64, :], ident[64:128, 64:128],
                             dstack[64:128, :], start=True, stop=True)
            nc.tensor.matmul(mv[64:128, :], ident[0:64, 0:64],
                             dstack[0:64, :], start=True, stop=True)
            recsb = psm.tile([128, QC], F32, tag="recsb")
            nc.vector.reciprocal(recsb[:], mv[:])
            otn = psm.tile([128, QC], F16, tag="otn")
            nc.vector.tensor_mul(otn[0:64, :], ot0[0:64, :], recsb[0:64, :])
            nc.vector.tensor_mul(otn[64:128, :], ot1[64:128, :], recsb[64:128, :])

            # -- O-projection (contract this core's 128 dmodel dims)
            for mt in range(NDT):
                op = pst.tile([128, QC], F32, tag="st", name="op")
                nc.tensor.matmul(op[:], wo_s[:, mt, :], otn[:],
                                 start=True, stop=True)
                oc = poc.tile([128, QC], F16, tag="oc")
                nc.vector.tensor_copy(oc[:], op[:])
                nc.sync.dma_start(outT[mt, qc, :, :], oc[:])


_PROGRAM = None


def _declare_io(nc):
    mk = lambda n, s, kind, dt=F16: nc.dram_tensor(n, list(s), dt, kind=kind).ap()
    blk4 = (NDT, NQC, 128, QC)
    ins = [mk("qT", (D, L), "ExternalInput"), mk("kT", (D, L), "ExternalInput"),
           mk("vT", (D, L), "ExternalInput"),
           mk("wq", (128, NDT, 128), "ExternalInput"),
           mk("wk", (128, NDT, 128), "ExternalInput"),
           mk("wv", (128, NDT, 128), "ExternalInput"),
           mk("wo", (128, NDT, 128), "ExternalInput")]
    outs = [mk("outT", blk4, "ExternalOutput")]
    return ins, outs


def _build_program(reps=1):
    global _PROGRAM
    if _PROGRAM is not None and reps == 1:
        return _PROGRAM
    nc = bacc.Bacc("TRN2", target_bir_lowering=False, debug=False,
                   num_devices=N_CORES)
    ins, outs = _declare_io(nc)
    with tile.TileContext(nc) as tc:
        _mha_core_kernel(tc, outs, ins, reps=reps)
    nc.compile()
    if reps == 1:
        _PROGRAM = nc
    return nc


def _tile_T(x):
    """[L, D] -> transposed [D, L] contiguous fp16."""
    return np.ascontiguousarray(x.T.astype(np.float16))


def _tile_w(w_slice):
    """[128, D] (rows = this core's dims) -> lhsT layout [128, NDT, 128]."""
    # lhsT[p, t, m] = w_slice[m, t*128+p]
    return np.ascontiguousarray(
        w_slice.reshape(128, NDT, 128).transpose(2, 1, 0).astype(np.float16))


def make_in_maps(query, key, value, w_q, w_k, w_v, w_o):
    qT = _tile_T(query.reshape(L, D))
    kT = _tile_T(key.reshape(L, D))
    vT = _tile_T(value.reshape(L, D))
    in_maps = []
    for c in range(N_CORES):
        sl = slice(c * 128, (c + 1) * 128)
        # O-proj lhsT: wo_t[d, t, m] = w_o[t*128+m, c*128+d]
        wo_t = np.ascontiguousarray(
            w_o[:, sl].reshape(NDT, 128, 128).transpose(2, 0, 1).astype(np.float16))
        in_maps.append({
            "qT": qT, "kT": kT, "vT": vT,
            "wq": _tile_w(w_q[sl]),
            "wk": _tile_w(w_k[sl]),
            "wv": _tile_w(w_v[sl]),
            "wo": wo_t,
        })
    return in_maps


def gather_out(results):
    """Sum per-core pre-tiled partials and restore [1, L, D]."""
    acc = results[0]["outT"].astype(np.float32)
    for c in range(1, N_CORES):
        acc += results[c]["outT"].astype(np.float32)
    # acc[t, qc, p, j] = out.T[t*128+p, qc*512+j] = out[qc*512+j, t*128+p]
    out = acc.transpose(1, 3, 0, 2).reshape(L, D)
    return np.ascontiguousarray(out).reshape(1, L, D)


def run(in_maps, trace=False):
    nc = _build_program()
    return run_bass_kernel_spmd(nc, in_maps, core_ids=list(range(N_CORES)),
                                trace=trace)


def kernel(query, key, value, w_q, w_k, w_v, w_o):
    query = np.asarray(query, dtype=np.float32)
    key = np.asarray(key, dtype=np.float32)
    value = np.asarray(value, dtype=np.float32)
    w_q = np.asarray(w_q, dtype=np.float32)
    w_k = np.asarray(w_k, dtype=np.float32)
    w_v = np.asarray(w_v, dtype=np.float32)
    w_o = np.asarray(w_o, dtype=np.float32)

    res = run(make_in_maps(query, key, value, w_q, w_k, w_v, w_o))
    return gather_out(res.results)


# revision 4
# speedup vs baseline: 1.3014x; 1.3014x over previous
"""Multi-head attention (B=1, L=4096, D=1024, H=16, d_k=64) on 8 TRN2 NeuronCores.

Sharding: head/tensor parallel. Core c owns heads 2c, 2c+1 (128 dmodel dims):
its slices of the QKV projection weights, full-L attention for its 2 heads,
and the partial O-projection for its 128-dim slice. Host sums the 8 partial
outputs (the allreduce of row-sharded tensor parallelism, done at gather).

v2 vs v1:
- ALL matmuls fp16 (v1 ran attention in f32r -> fp32_mode=HIGH on the PE,
  which power-throttled HAM to 1.2 GHz for ~440us of the run, and disabled
  FWL so every LDWEIGHTS cost ~210ns serialized). fp16 keeps K=8/8, enables
  FWL (hidden weight loads), and halves SBUF/stream energy.
- Score matmuls for the two heads are emitted adjacently as 64x128 row-tiles
  (T0/T8) so they co-issue and stream concurrently: 2x on scores.
- AV stationary is [V_h | ones64] ([ones64 | V_h] for head 1), so the softmax
  denominator comes out of the AV matmul already broadcast across 64
  partitions (the array columns were idle anyway). Normalization is then
  full-width DVE ops; no [1,512] reciprocal, no gpsimd broadcast.
- The two denominator blocks land in opposite partition halves from their AV
  blocks; two tiny 64x64 identity matmuls (T8/T2) swap the halves on the PE.
- O-projection contracts the full 128 local dims in one matmul per
  (dmodel-tile, q-chunk) instead of two 64-contractions.
- exp tiles are [128, 3*512] (GK=3) to amortize the ACT engine's ~352-cycle
  per-instruction overhead; scalar-engine exp is the critical path (~250us).
- fp16 output (halves the output DMA); host gathers/sums in fp32.
"""
import sys
if '/opt/trn_rl_repo' not in sys.path:
    sys.path.insert(0, '/opt/trn_rl_repo')

import numpy as np
from contextlib import ExitStack

import concourse.bass as bass
import concourse.tile as tile
from concourse import bacc, mybir
from concourse._compat import with_exitstack
from concourse.bass_utils import run_bass_kernel_spmd
from concourse.masks import make_identity

F32 = mybir.dt.float32
F16 = mybir.dt.float16
AF = mybir.ActivationFunctionType

N_CORES = 8
L = 4096
D = 1024
QC = 512                 # q-chunk (free dim of score/AV/O matmuls)
NQC = L // QC            # 8
NKT = L // 128           # 32 k-tiles
NDT = D // 128           # 8 dmodel tiles
GK = 3                   # k-tiles per exp group ([128, GK*QC] ACT tiles)
SCALE = 0.125            # 1/sqrt(d_k)


def _groups():
    """Split NKT k-tiles into groups of GK (last group ragged)."""
    gs, k0 = [], 0
    while k0 < NKT:
        nk = min(GK, NKT - k0)
        gs.append((k0, nk))
        k0 += nk
    return gs


@with_exitstack
def _mha_core_kernel(ctx, tc, outs, ins, reps=1):
    sb = ctx.enter_context(tc.tile_pool(name="sb", bufs=1))
    for _rep in range(reps):
        _mha_body(tc, sb, outs, ins)


def _mha_body(tc, sb, outs, ins):
    nc = tc.nc
    (outT,) = outs           # [NDT, NQC, 128, QC] fp16
    qT, kT, vT, wq, wk, wv, wo = ins
    # qT/kT/vT: [D, L] fp16 transposed activations
    # wq/wk/wv: [128, NDT, 128] fp16 (lhsT per ktile)
    # wo:       [128, NDT, 128] fp16 (lhsT per mtile; rows = local dims)

    # ---- weights (single contiguous DMAs) ----
    wq_s = sb.tile([128, NDT, 128], F16, tag="wq")
    wk_s = sb.tile([128, NDT, 128], F16, tag="wk")
    wv_s = sb.tile([128, NDT, 128], F16, tag="wv")
    wo_s = sb.tile([128, NDT, 128], F16, tag="wo")
    for w_s, w_d in ((wq_s, wq), (wk_s, wk), (wv_s, wv), (wo_s, wo)):
        nc.sync.dma_start(w_s[:], w_d[:])

    ident = sb.tile([128, 128], F16, tag="ident")
    make_identity(nc, ident[:])

    # ---- persistent activations (all fp16) ----
    QT_s = sb.tile([128, L], F16, tag="QT")
    KT_s = sb.tile([128, L], F16, tag="KT")
    VT_s = sb.tile([128, L], F16, tag="VT")
    # Vaug per ktile: cols [0:64]=V_h0, [64:128]=1.0  (head0 aug = [V0|1])
    #                 cols [128:192]=1.0, [192:256]=V_h1 (head1 aug = [1|V1])
    # => AV_h0 lands in psum rows 0:64 with d0 broadcast in rows 64:128;
    #    AV_h1 lands in rows 64:128 with d1 broadcast in rows 0:64.
    Vaug = sb.tile([128, NKT, 256], F16, tag="Vaug")
    nc.vector.memset(Vaug[:, :, 64:192], 1.0)

    # ========== phase 1: K, V (+transpose), Q projections ==========
    # Q is projected last so the phase-1 -> phase-2 junction is dense PE work
    # (no long PE-idle gap that would drop the HAM clock to 1.2 GHz).
    with (
        tc.tile_pool(name="xblk", bufs=3) as xblk,
        tc.tile_pool(name="pacc", bufs=1, space="PSUM") as pacc,
    ):
        def proj(dst, w_s, src_d):
            accs = [pacc.tile([128, QC], F32, tag=f"acc{qc}", name=f"acc{qc}")
                    for qc in range(NQC)]
            for t in range(NDT):
                blk = xblk.tile([128, L], F16, tag="blk")
                nc.sync.dma_start(blk[:], src_d[t * 128:(t + 1) * 128, :])
                for qc in range(NQC):
                    nc.tensor.matmul(accs[qc][:], w_s[:, t, :],
                                     blk[:, qc * QC:(qc + 1) * QC],
                                     start=(t == 0), stop=(t == NDT - 1))
            for qc in range(NQC):
                nc.vector.tensor_copy(dst[:, qc * QC:(qc + 1) * QC], accs[qc][:])

        proj(KT_s, wk_s, kT)
        proj(VT_s, wv_s, vT)

    # transpose VT -> Vaug rows (L-major), heads split around the ones blocks
    with tc.tile_pool(name="tps", bufs=2, space="PSUM") as tps:
        for rt in range(NKT):
            tp = tps.tile([128, 128], F16, tag="tp")
            nc.tensor.transpose(tp[:], VT_s[:, rt * 128:(rt + 1) * 128], ident[:])
            nc.vector.tensor_copy(Vaug[:, rt, 0:64], tp[:, 0:64])
            nc.vector.tensor_copy(Vaug[:, rt, 192:256], tp[:, 64:128])

    with (
        tc.tile_pool(name="xblk2", bufs=3) as xblk2,
        tc.tile_pool(name="pacc2", bufs=1, space="PSUM") as pacc2,
    ):
        accs = [pacc2.tile([128, QC], F32, tag=f"qacc{qc}", name=f"qacc{qc}")
                for qc in range(NQC)]
        for t in range(NDT):
            blk = xblk2.tile([128, L], F16, tag="blk")
            nc.sync.dma_start(blk[:], qT[t * 128:(t + 1) * 128, :])
            for qc in range(NQC):
                nc.tensor.matmul(accs[qc][:], wq_s[:, t, :],
                                 blk[:, qc * QC:(qc + 1) * QC],
                                 start=(t == 0), stop=(t == NDT - 1))
        for qc in range(NQC):
            nc.vector.tensor_copy(QT_s[:, qc * QC:(qc + 1) * QC], accs[qc][:])

    # ====== phase 2: per q-chunk: attention + O-proj ======
    # Software-pipelined: the normalize + O-projection of q-chunk qc-1 are
    # emitted interleaved with the scores/exp of q-chunk qc, so the serial
    # normalize chain never stalls the in-order PE queue (a >3.4us PE gap
    # drops the HAM clock to 1.2 GHz, and the bursty steady state can never
    # re-warm it). The AV matmuls lag their scores by 2 groups for the same
    # reason. The swapped denominators are matmul'd into the dead halves of
    # the ot banks so no extra PSUM bank is needed.
    groups = _groups()
    ngroups = len(groups)
    with (
        tc.tile_pool(name="pst", bufs=2, space="PSUM") as pst,
        tc.tile_pool(name="pot0", bufs=1, space="PSUM") as pot0,
        tc.tile_pool(name="pot1", bufs=1, space="PSUM") as pot1,
        tc.tile_pool(name="pat", bufs=4) as pat,
        tc.tile_pool(name="psm", bufs=2) as psm,
        tc.tile_pool(name="poc", bufs=2) as poc,
    ):
        def emit_scores_act(qc, k0, nk):
            q0, q1 = qc * QC, (qc + 1) * QC
            st0 = pst.tile([128, GK * QC], F32, tag="st", name="st0")
            st1 = pst.tile([128, GK * QC], F32, tag="st", name="st1")
            for j in range(nk):
                ks = slice((k0 + j) * 128, (k0 + j + 1) * 128)
                js = slice(j * QC, (j + 1) * QC)
                nc.tensor.matmul(st0[:, js], KT_s[0:64, ks],
                                 QT_s[0:64, q0:q1], start=True, stop=True)
                nc.tensor.matmul(st1[:, js], KT_s[64:128, ks],
                                 QT_s[64:128, q0:q1], start=True, stop=True)
            gs = slice(0, nk * QC)
            at0 = pat.tile([128, GK * QC], F16, tag="at0")
            nc.scalar.activation(at0[:, gs], st0[:, gs], AF.Exp, scale=SCALE)
            at1 = pat.tile([128, GK * QC], F16, tag="at1")
            nc.scalar.activation(at1[:, gs], st1[:, gs], AF.Exp, scale=SCALE)
            return at0, at1

        def emit_av(ot0, ot1, at0, at1, k0, nk):
            for j in range(nk):
                kt = k0 + j
                js = slice(j * QC, (j + 1) * QC)
                nc.tensor.matmul(ot0[:], Vaug[:, kt, 0:128], at0[:, js],
                                 start=(kt == 0), stop=(kt == NKT - 1))
                nc.tensor.matmul(ot1[:], Vaug[:, kt, 128:256], at1[:, js],
                                 start=(kt == 0), stop=(kt == NKT - 1))

        def emit_norm_a(pv):
            # copy denominators out, then matmul each into the dead half of
            # the OTHER head's ot bank (64x64 identity row/col tiles).
            ot0, ot1 = pv["ot0"], pv["ot1"]
            dstack = psm.tile([128, QC], F16, tag="dstack")
            nc.vector.tensor_copy(dstack[64:128, :], ot0[64:128, :])   # d0
            nc.vector.tensor_copy(dstack[0:64, :], ot1[0:64, :])       # d1
            nc.tensor.matmul(ot1[0:64, :], ident[64:128, 64:128],
                             dstack[64:128, :], start=True, stop=True)  # d0
            nc.tensor.matmul(ot0[64:128, :], ident[0:64, 0:64],
                             dstack[0:64, :], start=True, stop=True)    # d1
            pv["dstack"] = dstack

        def emit_norm_b(pv):
            ot0, ot1 = pv["ot0"], pv["ot1"]
            recsb = psm.tile([128, QC], F32, tag="recsb")
            nc.vector.reciprocal(recsb[0:64, :], ot1[0:64, :])      # 1/d0
            nc.vector.reciprocal(recsb[64:128, :], ot0[64:128, :])  # 1/d1
            otn = psm.tile([128, QC], F16, tag="otn")
            nc.vector.tensor_mul(otn[0:64, :], ot0[0:64, :], recsb[0:64, :])
            nc.vector.tensor_mul(otn[64:128, :], ot1[64:128, :], recsb[64:128, :])
            pv["otn"] = otn

        def emit_oproj(pv, mts):
            otn, qcp = pv["otn"], pv["qc"]
            for mt in mts:
                op = pst.tile([128, QC], F32, tag="st", name="op")
                nc.tensor.matmul(op[:], wo_s[:, mt, :], otn[:],
                                 start=True, stop=True)
                oc = poc.tile([128, QC], F16, tag="oc")
                nc.vector.tensor_copy(oc[:], op[:])
                nc.sync.dma_start(outT[mt, qcp, :, :], oc[:])

        prev = None
        for qc in range(NQC):
            ot0 = pot0.tile([128, QC], F32, tag="ot0")
            ot1 = pot1.tile([128, QC], F32, tag="ot1")
            ats = []
            for gi, (k0, nk) in enumerate(groups):
                ats.append(emit_scores_act(qc, k0, nk))
                if prev is not None:
                    if gi == 0:
                        emit_norm_a(prev)
                    elif gi == 1:
                        emit_norm_b(prev)
                    elif 3 <= gi <= 6:
                        emit_oproj(prev, [2 * (gi - 3), 2 * (gi - 3) + 1])
                if gi >= 2:
                    a0, a1 = ats[gi - 2]
                    emit_av(ot0, ot1, a0, a1, *groups[gi - 2])
            for gi in (ngroups - 2, ngroups - 1):
                a0, a1 = ats[gi]
                emit_av(ot0, ot1, a0, a1, *groups[gi])
            prev = {"qc": qc, "ot0": ot0, "ot1": ot1}
        emit_norm_a(prev)
        emit_norm_b(prev)
        emit_oproj(prev, range(NDT))


_PROGRAM = None


def _declare_io(nc):
    mk = lambda n, s, kind, dt=F16: nc.dram_tensor(n, list(s), dt, kind=kind).ap()
    blk4 = (NDT, NQC, 128, QC)
    ins = [mk("qT", (D, L), "ExternalInput"), mk("kT", (D, L), "ExternalInput"),
           mk("vT", (D, L), "ExternalInput"),
           mk("wq", (128, NDT, 128), "ExternalInput"),
           mk("wk", (128, NDT, 128), "ExternalInput"),
           mk("wv", (128, NDT, 128), "ExternalInput"),
           mk("wo", (128, NDT, 128), "ExternalInput")]
    outs = [mk("outT", blk4, "ExternalOutput")]
    return ins, outs


def _build_program(reps=1):
    global _PROGRAM
    if _PROGRAM is not None and reps == 1:
        return _PROGRAM
    nc = bacc.Bacc("TRN2", target_bir_lowering=False, debug=False,
                   num_devices=N_CORES)
    ins, outs = _declare_io(nc)
    with tile.TileContext(nc) as tc:
        _mha_core_kernel(tc, outs, ins, reps=reps)
    nc.compile()
    if reps == 1:
        _PROGRAM = nc
    return nc


def _tile_T(x):
    """[L, D] -> transposed [D, L] contiguous fp16."""
    return np.ascontiguousarray(x.T.astype(np.float16))


def _tile_w(w_slice):
    """[128, D] (rows = this core's dims) -> lhsT layout [128, NDT, 128]."""
    # lhsT[p, t, m] = w_slice[m, t*128+p]
    return np.ascontiguousarray(
        w_slice.reshape(128, NDT, 128).transpose(2, 1, 0).astype(np.float16))


def make_in_maps(query, key, value, w_q, w_k, w_v, w_o):
    qT = _tile_T(query.reshape(L, D))
    kT = _tile_T(key.reshape(L, D))
    vT = _tile_T(value.reshape(L, D))
    in_maps = []
    for c in range(N_CORES):
        sl = slice(c * 128, (c + 1) * 128)
        # O-proj lhsT: wo_t[d, t, m] = w_o[t*128+m, c*128+d]
        wo_t = np.ascontiguousarray(
            w_o[:, sl].reshape(NDT, 128, 128).transpose(2, 0, 1).astype(np.float16))
        in_maps.append({
            "qT": qT, "kT": kT, "vT": vT,
            "wq": _tile_w(w_q[sl]),
            "wk": _tile_w(w_k[sl]),
            "wv": _tile_w(w_v[sl]),
            "wo": wo_t,
        })
    return in_maps


def gather_out(results):
    """Sum per-core pre-tiled partials and restore [1, L, D]."""
    acc = results[0]["outT"].astype(np.float32)
    for c in range(1, N_CORES):
        acc += results[c]["outT"].astype(np.float32)
    # acc[t, qc, p, j] = out.T[t*128+p, qc*512+j] = out[qc*512+j, t*128+p]
    out = acc.transpose(1, 3, 0, 2).reshape(L, D)
    return np.ascontiguousarray(out).reshape(1, L, D)


def run(in_maps, trace=False):
    nc = _build_program()
    return run_bass_kernel_spmd(nc, in_maps, core_ids=list(range(N_CORES)),
                                trace=trace)


def kernel(query, key, value, w_q, w_k, w_v, w_o):
    query = np.asarray(query, dtype=np.float32)
    key = np.asarray(key, dtype=np.float32)
    value = np.asarray(value, dtype=np.float32)
    w_q = np.asarray(w_q, dtype=np.float32)
    w_k = np.asarray(w_k, dtype=np.float32)
    w_v = np.asarray(w_v, dtype=np.float32)
    w_o = np.asarray(w_o, dtype=np.float32)

    res = run(make_in_maps(query, key, value, w_q, w_k, w_v, w_o))
    return gather_out(res.results)
